# revision 1
# baseline (speedup 1.0000x reference)
"""Chamfer distance (squared L2) Bass kernel for Trainium2, 8 NeuronCores.

Problem: xyz1 [8, 8192, 3], xyz2 [8, 8192, 3] fp32.
  d[b,n,m] = ||xyz1[b,n] - xyz2[b,m]||^2
  out = mean_n min_m d + mean_m min_n d   (scalar)

Sharding: batch b -> core b (8 batches, 8 cores).

Device algorithm (per core):
  The pairwise distance tile is computed on the TensorEngine as a K=13
  augmented matmul in fp16 with hi/lo splitting for fp32-grade accuracy:
      d[n,m] = x.z + |x|^2 + |y|^2          (z = -2y)
  where each fp32 factor is split into fp16 hi+lo (lo scaled by 2^11 and
  the paired operand scaled by 2^-11 to stay in fp16 normal range).
  PSUM accumulates in fp32. ScalarE drains PSUM -> SBUF as fp16.
  VectorE maintains two running elementwise mins (fp16, 2x mode):
    - acc1[128,1024]: min over m-groups for the current n-tile  -> dist1
    - rmin2[128,8192]: min over n-tiles for every m             -> dist2
  dist2's final min over the partition axis is done by PE transposes of
  rmin2 in 128x128 chunks + free-axis reduce_min. Sums are reduced
  on-device to a single scalar per core.
"""

import numpy as np

B = 8
N = 8192  # points in xyz1 per batch
M = 8192  # points in xyz2 per batch
P = 128   # partitions
NT = N // P      # 64 n-tiles
GW = 1024        # m-group width (2 PSUM banks)
NG = M // GW     # 8 m-groups
K = 13           # augmented contraction dim
SPLIT = 2048.0   # 2^11 lo-component scale

_COMPILED = {}


def _build_nc():
    import concourse.mybir as mybir
    import concourse.tile as tile
    from concourse import bacc
    from concourse.bass_isa import ReduceOp
    from concourse.masks import make_identity

    f16 = mybir.dt.float16
    f32 = mybir.dt.float32
    MIN = mybir.AluOpType.min
    ADD = mybir.AluOpType.add
    X = mybir.AxisListType.X

    nc = bacc.Bacc("TRN2", target_bir_lowering=False, debug=False, num_devices=B)
    lhs_d = nc.dram_tensor("lhs", [K, N], f16, kind="ExternalInput").ap()
    rhs_d = nc.dram_tensor("rhs", [K, M], f16, kind="ExternalInput").ap()
    out_d = nc.dram_tensor("out", [1, 1], f32, kind="ExternalOutput").ap()

    with tile.TileContext(nc) as tc:
        from contextlib import ExitStack

        with ExitStack() as ctx:
            cpool = ctx.enter_context(tc.tile_pool(name="const", bufs=1))
            dpool = ctx.enter_context(tc.tile_pool(name="d16", bufs=4))
            pspool = ctx.enter_context(tc.tile_pool(name="ps", bufs=3, space="PSUM"))
            ptpool = ctx.enter_context(tc.tile_pool(name="pst", bufs=2, space="PSUM"))

            lhs = cpool.tile([K, N], f16)
            rhs = cpool.tile([K, M], f16)
            nc.sync.dma_start(lhs[:], lhs_d[:])
            nc.sync.dma_start(rhs[:], rhs_d[:])

            ident = cpool.tile([P, P], f16)
            make_identity(nc, ident[:])

            rmin2 = cpool.tile([P, M], f16)
            acc1 = cpool.tile([P, GW], f16)
            rmin1 = cpool.tile([P, NT], f16)
            d2mins = cpool.tile([P, NT], f16)

            for nt in range(NT):
                lhsT = lhs[:, nt * P:(nt + 1) * P]
                for g in range(NG):
                    ps = pspool.tile([P, GW], f32)
                    nc.tensor.matmul(
                        ps[:, 0:512], lhsT, rhs[:, g * GW:g * GW + 512],
                        start=True, stop=True)
                    nc.tensor.matmul(
                        ps[:, 512:GW], lhsT, rhs[:, g * GW + 512:(g + 1) * GW],
                        start=True, stop=True)
                    d16 = dpool.tile([P, GW], f16)
                    nc.scalar.copy(d16[:], ps[:])
                    # dist1: accumulate min over m-groups for this n-tile
                    if g == 0:
                        nc.vector.tensor_copy(acc1[:], d16[:])
                    else:
                        nc.vector.tensor_tensor(acc1[:], acc1[:], d16[:], MIN)
                    # dist2: accumulate min over n-tiles for this m-group
                    sl = rmin2[:, g * GW:(g + 1) * GW]
                    if nt == 0:
                        nc.vector.tensor_copy(sl, d16[:])
                    else:
                        nc.vector.tensor_tensor(sl, sl, d16[:], MIN)
                nc.vector.tensor_reduce(
                    rmin1[:, nt:nt + 1], acc1[:], axis=X, op=MIN)

            # dist2: min over partition axis via PE transpose of 128x128 chunks
            for c in range(M // P):
                pt = ptpool.tile([P, P], f16)
                nc.tensor.transpose(pt[:], rmin2[:, c * P:(c + 1) * P], ident[:])
                nc.vector.tensor_reduce(
                    d2mins[:, c:c + 1], pt[:], axis=X, op=MIN)

            s1 = cpool.tile([P, 1], f32)
            s2 = cpool.tile([P, 1], f32)
            s12 = cpool.tile([P, 1], f32)
            nc.vector.tensor_reduce(s1[:], rmin1[:], axis=X, op=ADD)
            nc.vector.tensor_reduce(s2[:], d2mins[:], axis=X, op=ADD)
            nc.vector.tensor_tensor(s12[:], s1[:], s2[:], ADD)
            nc.gpsimd.partition_all_reduce(s12[:], s12[:], P, ReduceOp.add)
            nc.sync.dma_start(out_d[:], s12[0:1, :])

    nc.compile()
    return nc


def _prep_operands(xyz1: np.ndarray, xyz2: np.ndarray):
    """Build per-batch fp16 split-precision operand matrices LHS/RHS [13, 8192]."""
    f32 = np.float32
    f16 = np.float16
    x = np.ascontiguousarray(xyz1, dtype=f32)          # [B, N, 3]
    z = np.ascontiguousarray(-2.0 * xyz2, dtype=f32)   # [B, M, 3]

    def split(a):
        hi = a.astype(f16)
        lo_s = ((a - hi.astype(f32)) * SPLIT).astype(f16)
        return hi, lo_s

    xhi, xlo_s = split(x)
    zhi, zlo_s = split(z)
    xhi_s = (xhi.astype(f32) / SPLIT).astype(f16)
    zhi_s = (zhi.astype(f32) / SPLIT).astype(f16)

    x2 = np.square(xyz1.astype(np.float64)).sum(-1).astype(f32)  # [B, N]
    y2 = np.square(xyz2.astype(np.float64)).sum(-1).astype(f32)  # [B, M]
    x2hi, x2lo_s = split(x2)
    y2hi, y2lo_s = split(y2)

    ones = np.ones((B, N), dtype=f16)
    inv_s = np.full((B, N), 1.0 / SPLIT, dtype=f16)

    # row pairing (lhs_row k) . (rhs_row k) summed over k gives
    #   x.z + |x|^2 + |y|^2  =  |x|^2 + |y|^2 - 2 x.y
    LHS = np.stack([
        xhi[:, :, 0], xhi[:, :, 1], xhi[:, :, 2],
        xhi_s[:, :, 0], xhi_s[:, :, 1], xhi_s[:, :, 2],
        xlo_s[:, :, 0], xlo_s[:, :, 1], xlo_s[:, :, 2],
        x2hi, x2lo_s,
        ones, inv_s,
    ], axis=1)  # [B, 13, N]
    RHS = np.stack([
        zhi[:, :, 0], zhi[:, :, 1], zhi[:, :, 2],
        zlo_s[:, :, 0], zlo_s[:, :, 1], zlo_s[:, :, 2],
        zhi_s[:, :, 0], zhi_s[:, :, 1], zhi_s[:, :, 2],
        ones, inv_s,
        y2hi, y2lo_s,
    ], axis=1)  # [B, 13, M]
    return np.ascontiguousarray(LHS), np.ascontiguousarray(RHS)


def _run(xyz1, xyz2, trace=False):
    from concourse.bass_utils import run_bass_kernel_spmd

    if "nc" not in _COMPILED:
        _COMPILED["nc"] = _build_nc()
    nc = _COMPILED["nc"]

    LHS, RHS = _prep_operands(np.asarray(xyz1), np.asarray(xyz2))
    in_maps = [{"lhs": LHS[b], "rhs": RHS[b]} for b in range(B)]
    res = run_bass_kernel_spmd(nc, in_maps, list(range(B)), trace=trace)
    total = np.float64(0.0)
    for b in range(B):
        total += np.float64(res.results[b]["out"][0, 0])
    out = np.asarray(total / (B * N), dtype=np.float32)[()]
    return np.asarray(out), res


def kernel(xyz1: np.ndarray, xyz2: np.ndarray) -> np.ndarray:
    out, _ = _run(xyz1, xyz2, trace=False)
    return out
